# revision 38
# baseline (speedup 1.0000x reference)
"""Trainium2 Bass kernel for GCE-GNN LocalAggregator (gnn_message_passing).

Computes, for each batch b:
    h = embedding[inputs]                            # [N, D] gather
    e_k = leakyrelu((h * a_k) @ h.T, 0.2)            # k = 0..3
    alpha = softmax(where(adj == k+1, e_k, -inf))    # edge-type select
    out = alpha @ h

Sharding: data-parallel over batch B=512 across 8 cores (64 local batches).
The embedding table is replicated per core in DRAM as bf16; looked-up rows
are fetched with one 128-row indirect DMA per pair (HW supports exactly one
gather offset per destination partition row).

Per-core layout ("pair" g = 2 batches sharing 128 partitions as (u, i)):
  h2[p=(u,i), g, d]   gathered rows, bf16
  h_sw                partition-swapped copy of h2 (u halves exchanged) via
                      2 SBUF-SBUF DMAs per chunk; gives the out-matmul an
                      rhs whose j-rows sit at either partition base so the
                      lhsT/rhs contraction bases always match.
  hT[d, g, (u,i)]     32 paired PE transposes [128,128], ACT evacs
  S[d, k, g, (u,i)]   a_k-scaled hT (4 DVE tensor_scalar per chunk)
  e_ps[(u,i),(k,j)]   2 bf16 matmuls per pair (contract d=128, free 256)
  alpha               edge-type select: bulk DVE memset NEG + 4 DVE
                      copy_predicated per chunk over host-built masks
  x = exp(lrelu(alpha)): lrelu is one DVE scalar_tensor_tensor
      (max(0.2*z, z)) so ACT only ever loads the Exp table once; masked
      entries give exp(-2e8) = 0 so softmax rows need no -inf handling
  row sums + reciprocal (DVE) -> r_inv
  xT: 2-pair-merged PE transposes; out-matmuls contract j=64 (bf16); ACT
      evac applies scale=r_inv (fusing the softmax divide); one DMA per
      2 pairs to DRAM.

Chunks are processed front(t) then back(t) in order; the gather pace
(~1.4us per 128-row indirect DMA, SWDGE-fixed-cost bound) overlaps the
per-chunk compute pipeline.
"""

import os
import sys

import numpy as np
import ml_dtypes

for _p in ("/opt/trn_rl_repo",):
    if _p not in sys.path and os.path.isdir(_p):
        sys.path.insert(0, _p)

import concourse.bass as bass
import concourse.bacc as bacc
import concourse.tile as tile
from concourse import mybir
from concourse.bass_utils import run_bass_kernel_spmd

B, N, D, V = 512, 64, 128, 43098
NCORES = 8
BC = B // NCORES          # 64 local batches per core
NPAIR = BC // 2           # 32 pairs
NCH = 8                   # pipeline chunks
GCH = NPAIR // NCH        # 8 pairs per chunk
ALPHA = 0.2
NEG_BIG = -1.0e9          # exp(NEG_BIG * {1, 0.2}) == 0 in bf16/fp32

FP32 = mybir.dt.float32
BF16 = mybir.dt.bfloat16
INT32 = mybir.dt.int32
U8 = mybir.dt.uint8
AF = mybir.ActivationFunctionType
OP = mybir.AluOpType


def build_nc():
    nc = bacc.Bacc("TRN2", target_bir_lowering=False, debug=False)

    idx_d = nc.dram_tensor("idx", [128, NPAIR], INT32, kind="ExternalInput")
    msk_d = nc.dram_tensor("mask4", [128, 4 * NPAIR * N], U8, kind="ExternalInput")
    emb_d = nc.dram_tensor("emb", [V, D], BF16, kind="ExternalInput")
    attn_d = nc.dram_tensor("attnT", [D, 4], FP32, kind="ExternalInput")
    ident_d = nc.dram_tensor("ident", [128, 128], BF16, kind="ExternalInput")
    out_d = nc.dram_tensor("out", [BC, N, D], FP32, kind="ExternalOutput")

    with tile.TileContext(nc) as tc:
        with (
            tc.tile_pool(name="singles", bufs=1) as singles,
            tc.tile_pool(name="big", bufs=1) as big,
            tc.tile_pool(name="xtp", bufs=4) as xtp,
            tc.tile_pool(name="outp", bufs=4) as outp,
            tc.tile_pool(name="ps_h", bufs=2, space="PSUM") as ps_h,
            tc.tile_pool(name="ps_e", bufs=2, space="PSUM") as ps_e,
            tc.tile_pool(name="ps_x", bufs=2, space="PSUM") as ps_x,
            tc.tile_pool(name="ps_o", bufs=2, space="PSUM") as ps_o,
        ):
            # ---- constants / inputs ----
            idx_sb = singles.tile([128, NPAIR], INT32)
            nc.sync.dma_start(out=idx_sb[:, :], in_=idx_d[:, :])
            attn_sb = singles.tile([128, 4], FP32)
            nc.sync.dma_start(out=attn_sb[:, :], in_=attn_d[:, :])
            ident = singles.tile([128, 128], BF16)
            nc.sync.dma_start(out=ident[:, :], in_=ident_d[:, :])
            # host-precomputed edge-type masks: [p=(u,i), k, g, j] uint8
            msk_sb = big.tile([128, 4, NPAIR, N], U8, tag="msk")
            nc.sync.dma_start(
                out=msk_sb[:, :, :, :],
                in_=msk_d.ap().rearrange("p (k g j) -> p k g j", k=4, j=N),
            )

            # ---- persistent tensors ----
            h2 = big.tile([128, NPAIR, D], BF16, tag="h2")     # [(u,i), g, d]
            h_sw = big.tile([128, NPAIR, D], BF16, tag="hsw")  # u-halves swapped
            hT = big.tile([128, NPAIR, 128], BF16, tag="hT")   # [d, g, (u,i)]
            S = big.tile([128, 4, NPAIR, 128], BF16, tag="S")  # [d, k, g, (u,i)]
            e_all = big.tile([128, 4, NPAIR, N], BF16, tag="e")
            alpha = big.tile([128, NPAIR, N], BF16, tag="alpha")
            xs = big.tile([128, NPAIR, N], BF16, tag="xs")
            x = big.tile([128, NPAIR, N], BF16, tag="x")
            ssum = singles.tile([128, NPAIR], FP32)
            rinv = singles.tile([128, NPAIR], FP32)

            # select background (input-independent; runs in the head shadow)
            nc.vector.memset(alpha[:, :, :], NEG_BIG)

            # ---- gather: one 128-row indirect DMA per pair ----
            for g in range(NPAIR):
                nc.gpsimd.indirect_dma_start(
                    out=h2[:, g, :],
                    out_offset=None,
                    in_=emb_d[:, :],
                    in_offset=bass.IndirectOffsetOnAxis(
                        ap=idx_sb[:, g : g + 1], axis=0
                    ),
                )
            for t in range(NCH):
                gr = slice(t * GCH, (t + 1) * GCH)
                nc.sync.dma_start(out=h_sw[0:64, gr, :], in_=h2[64:128, gr, :])
                nc.sync.dma_start(out=h_sw[64:128, gr, :], in_=h2[0:64, gr, :])

            out_flat = out_d.ap().rearrange(
                "(t gg u) i d -> t (u i) gg d", gg=2, u=2
            )  # [16, 128, 2, 128]

            def front(t):
                gr = slice(t * GCH, (t + 1) * GCH)

                # hT: paired transposes, 2 pairs per PSUM tile + 1 evac
                for s in range(GCH // 2):
                    g0 = t * GCH + 2 * s
                    hps = ps_h.tile([128, 2, 128], BF16, tag="hT_ps")
                    for q in range(2):
                        nc.tensor.transpose(
                            out=hps[:, q, :],
                            in_=h2[:, g0 + q, :],
                            identity=ident[:, :],
                        )
                    nc.scalar.copy(out=hT[:, g0 : g0 + 2, :], in_=hps[:, :, :])

                # S = hT * a_k for this chunk
                for k in range(4):
                    nc.vector.tensor_scalar_mul(
                        out=S[:, k, gr, :].rearrange("p g ui -> p (g ui)"),
                        in0=hT[:, gr, :].rearrange("p g ui -> p (g ui)"),
                        scalar1=attn_sb[:, k : k + 1],
                    )

                # e matmuls + evac (alternate ACT / DVE for balance)
                for gi in range(GCH):
                    g = t * GCH + gi
                    eps = ps_e.tile([128, 4, N], FP32, tag="e_ps")
                    for u in range(2):
                        nc.tensor.matmul(
                            out=eps[u * 64 : (u + 1) * 64, :, :],
                            lhsT=hT[:, g, u * 64 : (u + 1) * 64],
                            rhs=S[:, :, g, u * 64 : (u + 1) * 64],
                            start=True,
                            stop=True,
                        )
                    nc.scalar.copy(out=e_all[:, :, g, :], in_=eps[:, :, :])

                # edge-type select into alpha (background NEG_BIG)
                for k in range(4):
                    nc.vector.copy_predicated(
                        out=alpha[:, gr, :],
                        mask=msk_sb[:, k, gr, :],
                        data=e_all[:, k, gr, :],
                    )

                # x = exp(leakyrelu(alpha)); lrelu = max(0.2*z, z) on DVE so
                # ACT only ever runs Exp (single activation-table load)
                flat = "p g j -> p (g j)"
                nc.vector.scalar_tensor_tensor(
                    out=xs[:, gr, :].rearrange(flat),
                    in0=alpha[:, gr, :].rearrange(flat),
                    scalar=ALPHA,
                    in1=alpha[:, gr, :].rearrange(flat),
                    op0=OP.mult,
                    op1=OP.max,
                )
                nc.scalar.activation(
                    out=x[:, gr, :].rearrange(flat),
                    in_=xs[:, gr, :].rearrange(flat),
                    func=AF.Exp,
                )

                # softmax denominators
                nc.vector.reduce_sum(
                    out=ssum[:, gr], in_=x[:, gr, :], axis=mybir.AxisListType.X
                )
                nc.vector.reciprocal(out=rinv[:, gr], in_=ssum[:, gr])

            def back(t):
                # x transposes (2 pairs each) + out matmuls + scaled evac + DMA
                for s in range(GCH // 2):
                    g0 = t * GCH + 2 * s
                    xps = ps_x.tile([128, 128], BF16, tag="xT_ps")
                    nc.tensor.transpose(
                        out=xps[:, :],
                        in_=x[:, g0 : g0 + 2, :].rearrange("p a j -> p (a j)"),
                        identity=ident[:, :],
                    )
                    xsb = xtp.tile([128, 128], BF16, tag="xT_sb")
                    nc.scalar.copy(out=xsb[:, :], in_=xps[:, :])
                    osb = outp.tile([128, 2, D], FP32, tag="o_sb")
                    for gg in range(2):
                        g = g0 + gg
                        ops = ps_o.tile([128, D], FP32, tag="o_ps")
                        for u in range(2):
                            base = gg * 64
                            rhs_t = h2 if u == gg else h_sw
                            nc.tensor.matmul(
                                out=ops[u * 64 : (u + 1) * 64, :],
                                lhsT=xsb[base : base + 64, u * 64 : (u + 1) * 64],
                                rhs=rhs_t[base : base + 64, g, :],
                                start=True,
                                stop=True,
                            )
                        nc.scalar.activation(
                            out=osb[:, gg, :],
                            in_=ops[:, :],
                            func=AF.Copy,
                            scale=rinv[:, g : g + 1],
                        )
                    nc.sync.dma_start(
                        out=out_flat[g0 // 2],
                        in_=osb[:, :, :],
                    )

            for t in range(NCH):
                front(t)
                back(t)
    nc.compile()
    return nc


_CACHE = {}


def _compiled():
    if "nc" not in _CACHE:
        _CACHE["nc"] = build_nc()
    return _CACHE["nc"]


def _shard_inputs(inputs, adj, embedding, attn_a):
    inputs = np.asarray(inputs)
    adj = np.asarray(adj)
    emb = np.ascontiguousarray(
        np.asarray(embedding, dtype=np.float32).astype(ml_dtypes.bfloat16)
    )
    attnT = np.ascontiguousarray(np.asarray(attn_a, dtype=np.float32).T)  # [D, 4]
    identity = np.ascontiguousarray(np.eye(128, dtype=np.float32).astype(ml_dtypes.bfloat16))
    in_maps = []
    for c in range(NCORES):
        sl = slice(c * BC, (c + 1) * BC)
        # idx2[(u,i), g] = inputs[c*BC + 2g+u, i]
        idx2 = np.ascontiguousarray(
            inputs[sl]
            .astype(np.int32)
            .reshape(NPAIR, 2, N)
            .transpose(1, 2, 0)
            .reshape(128, NPAIR)
        )
        adj_c = adj[sl].astype(np.int32)  # [BC, N, N]
        # [p=(u,i), (g, j)]
        adj_big = adj_c.reshape(NPAIR, 2, N, N).transpose(1, 2, 0, 3).reshape(
            128, NPAIR * N
        )
        # one-hot edge-type masks, [p, k, (g, j)] packed to [128, 4*NPAIR*N]
        mask4 = np.ascontiguousarray(
            np.stack([(adj_big == k + 1) for k in range(4)], axis=1)
            .astype(np.uint8)
            .reshape(128, 4 * NPAIR * N)
        )
        in_maps.append(dict(idx=idx2, mask4=mask4, emb=emb, attnT=attnT, ident=identity))
    return in_maps


def kernel(inputs, adj, mask_item, item, embedding, attn_a):
    in_maps = _shard_inputs(inputs, adj, embedding, attn_a)
    res = run_bass_kernel_spmd(
        _compiled(), in_maps, core_ids=list(range(NCORES))
    ).results
    out = np.concatenate([np.asarray(res[c]["out"]) for c in range(NCORES)], axis=0)
    return out.astype(np.float32)
